# revision 32
# baseline (speedup 1.0000x reference)
"""Multi-head causal attention (B=2, S=2048, D=1024, H=16) on 8 trn2 cores.

Sharding: core c handles batch b = c // 4 and head group g = c % 4 (4 heads,
256 feature columns). Each core computes its heads' attention context and a
partial output projection (ctx_g @ Wo[rows_g]); the host sums the 4 partials
per batch and adds bo.

Design (all matmul operands bf16; fp32 PSUM, denominators; bf16 output
partials summed on the host in fp32):
- Q^T head-pair-major [128, hm, S]; K^T zero-padded per head so score
  matmuls contract over K=128 (full-row stationaries keep Fast Weight Load
  enabled -- K=64 stationaries disable FWL and expose ~100ns LDWEIGHTS per
  matmul, measured +30us).
- Softmax normalization: denominator row (from a ones column folded into V)
  -> DMA partition-scatter [1,512]->[128,4] (a single-partition DVE
  reciprocal runs 1-lane at ~3.3us; scattered it is ~0.2us) -> bf16
  reciprocal -> DMA gather -> PE outer-product broadcast
  (ones[1,128] x recip[1,512], accumulating both heads into one PSUM tile)
  -> tensor_tensor multiply.  No DRAM bounce.
- The kernel is emitted as fine-grained units: attention (n, hm, ski) steps
  interleaved with projection / output-projection chunks so the exp (ACT)
  always has PE work to hide behind and the PE never idles >3.4us (HAM
  stays at full clock).  Units whose first instruction waits on the norm
  DMA chain (the PE broadcast) are re-emitted 10 units later so they don't
  block the in-order PE queue.
- PSUM drains on DVE (plus ACT Copy for the final output tiles); input and
  output DMAs are spread across the sync/scalar/gpsimd queues.
"""

import os
import sys
import types
from contextlib import ExitStack

import numpy as np
import ml_dtypes

import concourse.bacc as bacc
import concourse.bass as bass
import concourse.mybir as mybir
import concourse.tile as tile
from concourse.bass_utils import run_bass_kernel_spmd


def _install_ntff_hook():
    """The agent image's antenv lacks axon_hooks, so trn_boot's NTFF hook
    install degrades silently. Recreate the module + hook so trace=True works."""
    if "antenv.axon_hooks" in sys.modules:
        return
    try:
        mod = types.ModuleType("antenv.axon_hooks")
        holder = [None]
        mod.set_axon_ntff_profile_hook = lambda h: holder.__setitem__(0, h)
        mod.get_axon_ntff_profile_hook = lambda: holder[0]
        from trn_agent_boot.trn_boot import _ntff_profile_via_ctypes

        hook = _ntff_profile_via_ctypes("/opt/axon/libaxon_pjrt.so")
        if hook is None:
            return
        mod.set_axon_ntff_profile_hook(hook)
        sys.modules["antenv.axon_hooks"] = mod
    except Exception:
        pass

B, S, D, H, HD = 2, 2048, 1024, 16, 64
NCORES = 8
GROUPS = 4          # head groups (cores) per batch
HC = H // GROUPS    # heads per core
DG = HC * HD        # feature columns per core (256)
P = 128
KSUB = D // P       # 8 contraction subtiles for the projections
SQT = 512           # sq tile width (free dim of scores/ctx matmuls)
NSQ = S // SQT      # 4
NST = S // P        # 16 s subtiles of 128
F32 = mybir.dt.float32
F32R = mybir.dt.float32r
BF = mybir.dt.bfloat16
EXP = mybir.ActivationFunctionType.Exp

_CACHE = {}


def _mha_tile_kernel(tc, xT, wq, wk, wv, wo, out):
    nc = tc.nc
    scale = 1.0 / np.sqrt(np.float32(HD))

    with ExitStack() as ctx:
        consts = ctx.enter_context(tc.tile_pool(name="consts", bufs=1))
        # PSUM budget: pps 1-bank x2 + sps 2-bank x2 + cps 1-bank x2 = 8 banks
        pps = ctx.enter_context(tc.tile_pool(name="pps", bufs=2, space="PSUM"))
        sps = ctx.enter_context(tc.tile_pool(name="sps", bufs=2, space="PSUM"))
        cps = ctx.enter_context(tc.tile_pool(name="cps", bufs=2, space="PSUM"))
        xp = ctx.enter_context(tc.tile_pool(name="xp", bufs=3))
        ptp = ctx.enter_context(tc.tile_pool(name="ptp", bufs=7))
        smalls = ctx.enter_context(tc.tile_pool(name="smalls", bufs=4))
        scr = ctx.enter_context(tc.tile_pool(name="scr", bufs=4))
        outp = ctx.enter_context(tc.tile_pool(name="outp", bufs=3))

        # --- persistent SBUF tensors ---
        wq_sb = consts.tile([P, KSUB, DG], BF)
        wk_sb = consts.tile([P, KSUB, DG], BF)
        wv_sb = consts.tile([P, KSUB, DG], BF)
        wo_sb = consts.tile([P, DG // P, D], BF)
        # Q^T and K^T, head-pair-major: head 2*hm at [0:64, hm, :], head
        # 2*hm+1 at [64:128, hm, :].  Score matmuls are emitted as K=64
        # row-tiled pairs (tile_position (0,0) / (64,0)) so both heads'
        # scores compute CONCURRENTLY on disjoint row-groups of the PE
        # array, into separate PSUM banks.
        qt_sb = consts.tile([P, DG // P, S], BF)
        kt_sb = consts.tile([P, DG // P, S], BF)
        # V with the ones column baked in, per s-subtile and head:
        #   even h: [V(64) | 1 | 0(63)]  -> ctx rows 0-63, denom row 64
        #   odd  h: [1 | 0(63) | V(64)]  -> denom row 0, ctx rows 64-127
        v_sb = consts.tile([P, NST, HC, P], BF)
        ctxt_sb = consts.tile([P, DG // P, S], BF)  # normalized ctx^T, qt layout
        # recip-broadcast operands, kept K=128 so the broadcast matmuls don't
        # force a PE array-tiling mode switch (K=1 stationaries reconfigure
        # the array and serialize against in-flight matmuls): ones_sb has
        # partition 0 = ones over cols 0:64 and partition 1 = ones over cols
        # 64:128, so ONE matmul against recEO (recE on partition 0, recO on
        # partition 1, zeroed body so 0-weight x junk can't make NaNs)
        # broadcasts recE to out rows 0:64 and recO to rows 64:128.  Two
        # alternating recEO slots so consecutive norm chains don't WAR-stall.
        ones_sb = consts.tile([P, P], BF)
        recEO = consts.tile([P, 2, SQT], BF)  # [.., parity, ..]

        xts = []  # per-slice x tiles

        def emit_xdma(n):
            # one batched DMA per slice (a dma_start costs ~630ns of issue
            # time on its engine, so fewer+bigger wins; the transfer itself
            # fans out across all 16 DMA engines).  Slice 0 is split so the
            # first projection matmul only waits on the k=0 chunk.
            xn = xp.tile([P, KSUB, SQT], BF, tag="xT", bufs=3, name=f"xn_{n}")
            nsl = slice(n * SQT, (n + 1) * SQT)
            if n == 0:
                nc.sync.dma_start(out=xn[:, 0, :], in_=xT[:, 0, nsl])
                nc.gpsimd.dma_start(out=xn[:, 1:KSUB, :], in_=xT[:, 1:KSUB, nsl])
            else:
                nc.gpsimd.dma_start(out=xn, in_=xT[:, :, nsl])
            xts.append(xn)

        # first-needed DMAs first: the k=0 chunks of wq + x gate the first
        # matmul and ride sync (scalar's first issue waits ~1.3us for the
        # ACT table load; gpsimd's queue also starts late).  Remaining
        # weights ride scalar (ACT idle until the first exp), x gpsimd.
        nc.sync.dma_start(out=wq_sb[:, 0, :], in_=wq[:, 0, :])
        emit_xdma(0)
        nc.sync.dma_start(out=wk_sb[:, 0, :], in_=wk[:, 0, :])
        nc.scalar.dma_start(out=wq_sb[:, 1:KSUB, :], in_=wq[:, 1:KSUB, :])
        nc.scalar.dma_start(out=wk_sb[:, 1:KSUB, :], in_=wk[:, 1:KSUB, :])
        nc.scalar.dma_start(out=wv_sb, in_=wv)

        # zero/ones fills for the V padding
        nc.vector.memset(v_sb[:, :, 0:HC:2, HD:P], 0.0)
        nc.vector.memset(v_sb[:, :, 1:HC:2, 0:HD], 0.0)
        for h in range(HC):
            ones_col = 64 if h % 2 == 0 else 0
            nc.vector.memset(v_sb[:, :, h, ones_col : ones_col + 1], 1.0)
        nc.vector.memset(ones_sb, 0.0)
        nc.vector.memset(ones_sb[0:1, 0:64], 1.0)
        nc.vector.memset(ones_sb[32:33, 64:P], 1.0)
        nc.vector.memset(recEO, 0.0)

        def proj_units(n):
            """QKV projection chunks for x slice n: 8 independent units."""
            nsl = slice(n * SQT, (n + 1) * SQT)
            units = []

            def qchunk(m):
                def u():
                    xn = xts[n]
                    ps = pps.tile([P, SQT], F32, tag="p", name=f"qps_{n}_{m}")
                    for k in range(KSUB):
                        nc.tensor.matmul(
                            ps,
                            lhsT=wq_sb[:, k, m * P : (m + 1) * P],
                            rhs=xn[:, k, :],
                            start=(k == 0),
                            stop=(k == KSUB - 1),
                        )
                    nc.vector.tensor_copy(out=qt_sb[:, m, nsl], in_=ps)
                return u

            def kchunk(m):
                def u():
                    xn = xts[n]
                    ps = pps.tile([P, SQT], F32, tag="p", name=f"kps_{n}_{m}")
                    for k in range(KSUB):
                        nc.tensor.matmul(
                            ps,
                            lhsT=wk_sb[:, k, m * P : (m + 1) * P],
                            rhs=xn[:, k, :],
                            start=(k == 0),
                            stop=(k == KSUB - 1),
                        )
                    nc.vector.tensor_copy(out=kt_sb[:, m, nsl], in_=ps)
                return u

            def vchunk(sst):
                def u():
                    xn = xts[n]
                    st0 = n * (SQT // P)
                    ps = pps.tile([P, SQT], F32, tag="p", name=f"vps_{n}_{sst}")
                    for k in range(KSUB):
                        nc.tensor.matmul(
                            ps[:, 0:DG],
                            lhsT=xn[:, k, sst * P : (sst + 1) * P],
                            rhs=wv_sb[:, k, :],
                            start=(k == 0),
                            stop=(k == KSUB - 1),
                        )
                    psv = ps[:, 0:DG].rearrange("p (h d) -> p h d", h=HC, d=HD)
                    nc.vector.tensor_copy(
                        out=v_sb[:, st0 + sst, 0:HC:2, 0:HD], in_=psv[:, 0:HC:2, :]
                    )
                    nc.vector.tensor_copy(
                        out=v_sb[:, st0 + sst, 1:HC:2, HD:P], in_=psv[:, 1:HC:2, :]
                    )
                return u

            units.append(qchunk(0))
            units.append(qchunk(1))
            units.append(kchunk(0))
            units.append(kchunk(1))
            for sst in range(SQT // P):
                units.append(vchunk(sst))
            return units

        def attn_units(n):
            """Attention for sq-tile n: per head pair hm, one unit per PAIR
            of ski steps (4 row-tiled score matmuls + exps back-to-back,
            then the previous pair's 4 PV matmuls) so the PE array-tiling
            mode ((64,128) scores vs (128,128) everything else) switches
            once per unit instead of twice per step; plus norm units."""
            nski = 4 * n + 4
            sq0 = n * SQT
            nsl = slice(sq0, sq0 + SQT)
            units = []

            def emit_pv(state, pend, hm, nski):
                ski, w0, pt = pend
                nc.tensor.matmul(
                    state["cpsA"][:, w0:],
                    lhsT=v_sb[:, ski, 2 * hm, :],
                    rhs=pt[:, 0, w0:],
                    start=(ski == 0),
                    stop=(ski == nski - 1),
                )
                nc.tensor.matmul(
                    state["cpsB"][:, w0:],
                    lhsT=v_sb[:, ski, 2 * hm + 1, :],
                    rhs=pt[:, 1, w0:],
                    start=(ski == 0),
                    stop=(ski == nski - 1),
                )

            for hm in range(DG // P):
                state = {"pend": [], "cpsA": None, "cpsB": None,
                         "recs": None, "scrs": None}

                def score_step(ski, hm=hm, state=state, n=n, nski=nski, sq0=sq0):
                    if ski == 0:
                        state["cpsA"] = cps.tile(
                            [P, SQT], F32, tag="ctx", name=f"cA_{n}_{hm}"
                        )
                        state["cpsB"] = cps.tile(
                            [P, SQT], F32, tag="ctx", name=f"cB_{n}_{hm}"
                        )
                    diag = ski >= 4 * n
                    w0 = (128 * ski - sq0) if diag else 0
                    spsum = sps.tile(
                        [P, 2, SQT], F32, tag="s", name=f"s_{n}_{hm}_{ski}"
                    )
                    pt = ptp.tile(
                        [P, 2, SQT], BF, tag="pt", name=f"pt_{n}_{hm}_{ski}"
                    )
                    # the two heads' scores as a K=64 row-tiled pair:
                    # row-groups {0,1} / {2,3} compute concurrently
                    nc.tensor.matmul(
                        spsum[:, 0, w0:],
                        lhsT=kt_sb[0:64, hm, ski * P : (ski + 1) * P],
                        rhs=qt_sb[0:64, hm, sq0 + w0 : sq0 + SQT],
                        start=True,
                        stop=True,
                    )
                    nc.tensor.matmul(
                        spsum[:, 1, w0:],
                        lhsT=kt_sb[64:P, hm, ski * P : (ski + 1) * P],
                        rhs=qt_sb[64:P, hm, sq0 + w0 : sq0 + SQT],
                        start=True,
                        stop=True,
                    )
                    nc.scalar.activation(
                        out=pt[:, :, w0:], in_=spsum[:, :, w0:],
                        func=EXP, bias=0.0, scale=float(scale),
                    )
                    if diag:  # zero entries with sk > sq in the diag block
                        for j in (0, 1):
                            nc.gpsimd.affine_select(
                                out=pt[:, j, w0 : w0 + P],
                                in_=pt[:, j, w0 : w0 + P],
                                pattern=[[1, P]],
                                compare_op=mybir.AluOpType.is_ge,
                                fill=0.0,
                                base=0,
                                channel_multiplier=-1,
                            )
                    state["pend"].append((ski, w0, pt))

                def batch(j, hm=hm, state=state, nski=nski, score_step=score_step):
                    def u():
                        # scores first (64-row-tiled mode), then PV pairs
                        # from TWO units ago (128-mode) -- the deep lag keeps
                        # the PV inputs (exp+mask chain) ready well before
                        # scheduling so the Tile scheduler doesn't slot a PV
                        # between the concurrent score pair
                        score_step(j)
                        if j + 1 < nski:
                            score_step(j + 1)
                        while len(state["pend"]) > 4:
                            emit_pv(state, state["pend"].pop(0), hm, nski)
                    return u

                def normA(hm=hm, state=state, n=n, nski=nski):
                    def u():
                        while state["pend"]:
                            emit_pv(state, state["pend"].pop(0), hm, nski)
                        scrE = scr.tile([P, SQT], F32, tag="scr", name=f"scE_{n}_{hm}")
                        scrO = scr.tile([P, SQT], F32, tag="scr", name=f"scO_{n}_{hm}")
                        nc.vector.tensor_copy(out=scrE, in_=state["cpsA"])
                        nc.vector.tensor_copy(out=scrO, in_=state["cpsB"])
                        # scatter the denom rows across partitions: a [1, 512]
                        # DVE op runs on one lane (~3.3us); [128, 4] is ~26ns
                        sprE = smalls.tile([P, SQT // P], F32, tag="spr", name=f"spE_{n}_{hm}")
                        sprO = smalls.tile([P, SQT // P], F32, tag="spr", name=f"spO_{n}_{hm}")
                        nc.sync.dma_start(out=sprE, in_=scrE[64:65, :])
                        nc.sync.dma_start(out=sprO, in_=scrO[0:1, :])
                        state["scrs"] = (scrE, scrO)
                        state["sprs"] = (sprE, sprO)
                    return u

                def normA2(hm=hm, state=state, n=n):
                    def u():
                        sprE, sprO = state["sprs"]
                        rbE = smalls.tile([P, SQT // P], BF, tag="sprb", name=f"rbE_{n}_{hm}")
                        rbO = smalls.tile([P, SQT // P], BF, tag="sprb", name=f"rbO_{n}_{hm}")
                        with nc.allow_low_precision(reason="bf16 softmax denom"):
                            nc.vector.reciprocal(out=rbE, in_=sprE)
                            nc.vector.reciprocal(out=rbO, in_=sprO)
                        # gather into partitions 0/1 of the (zero-bodied)
                        # recEO slot; the broadcast matmul stays K=128
                        par = (2 * n + hm) % 2
                        nc.sync.dma_start(out=recEO[0:1, par, :], in_=rbE)
                        nc.sync.dma_start(out=recEO[32:33, par, :], in_=rbO)
                        state["recs"] = par
                    return u

                def normB(hm=hm, state=state, n=n, nsl=nsl):
                    def u():
                        scrE, scrO = state["scrs"]
                        par = state["recs"]
                        bps = pps.tile([P, SQT], F32, tag="p", name=f"bc_{n}_{hm}")
                        nc.tensor.matmul(
                            bps, lhsT=ones_sb, rhs=recEO[:, par, :],
                            start=True, stop=True,
                        )
                        nc.vector.tensor_tensor(
                            ctxt_sb[0:64, hm, nsl], scrE[0:64, :], bps[0:64, :],
                            mybir.AluOpType.mult,
                        )
                        nc.vector.tensor_tensor(
                            ctxt_sb[64:P, hm, nsl], scrO[64:P, :], bps[64:P, :],
                            mybir.AluOpType.mult,
                        )
                    return u

                for j in range(0, nski, 2):
                    units.append(batch(j))
                units.append(normA())
                units.append(normA2())
                nb = normB()
                units.append(lambda nb=nb: defer(nb, 6))
            return units

        def outproj_units(n, act_copies=False):
            """Partial output projection chunks for st tiles 4n..4n+3."""
            units = []
            ots = {}

            def chunk(st, nn):
                def u():
                    if nn == 0:
                        ots[st] = outp.tile([P, D], BF, tag="out", name=f"ot_{st}")
                    ot = ots[st]
                    ps = pps.tile([P, SQT], F32, tag="p", name=f"ops_{st}_{nn}")
                    for k in range(DG // P):
                        nc.tensor.matmul(
                            ps,
                            lhsT=ctxt_sb[:, k, st * P : (st + 1) * P],
                            rhs=wo_sb[:, k, nn * SQT : (nn + 1) * SQT],
                            start=(k == 0),
                            stop=(k == DG // P - 1),
                        )
                    if act_copies and (st + nn) % 2 == 0:
                        nc.scalar.activation(
                            out=ot[:, nn * SQT : (nn + 1) * SQT], in_=ps,
                            func=mybir.ActivationFunctionType.Copy,
                        )
                    else:
                        nc.vector.tensor_copy(
                            out=ot[:, nn * SQT : (nn + 1) * SQT], in_=ps
                        )
                    eng = (nc.sync, nc.gpsimd)[(st + nn) % 2]
                    eng.dma_start(
                        out=out[st * P : (st + 1) * P, nn * SQT : (nn + 1) * SQT],
                        in_=ot[:, nn * SQT : (nn + 1) * SQT],
                    )
                return u

            for st in range(4 * n, 4 * n + 4):
                for nn in range(D // SQT):
                    units.append(chunk(st, nn))
            return units

        deferred = []  # [(countdown, fn)] -- emitted a few units later so a
        # unit whose first instruction waits on a long non-PE chain (the norm
        # reciprocal's DMA scatter/gather) doesn't block the in-order PE queue

        def emit(u):
            u()
            for d in deferred:
                d[0] -= 1
            while deferred and deferred[0][0] <= 0:
                deferred.pop(0)[1]()

        def defer(fn, after):
            deferred.append([after, fn])

        def flush_deferred():
            while deferred:
                deferred.pop(0)[1]()

        def interleave(steps, fillers, reserve=0):
            """Emit steps with fillers spread evenly between them; the last
            `reserve` fillers are held back until after all steps (PE work to
            hide the final norm-chain latency)."""
            nf, ns = len(fillers) - reserve, len(steps)
            fi = 0
            for i, u in enumerate(steps):
                emit(u)
                want = (i + 1) * nf // ns
                while fi < want:
                    emit(fillers[fi])
                    fi += 1
            while fi < len(fillers):
                emit(fillers[fi])
                fi += 1

        # --- schedule ---
        emit_xdma(1)
        for u in proj_units(0):
            emit(u)
        emit_xdma(2)
        interleave(attn_units(0), proj_units(1))
        emit_xdma(3)
        interleave(attn_units(1), proj_units(2))
        nc.sync.dma_start(out=wo_sb, in_=wo)
        interleave(attn_units(2), proj_units(3) + outproj_units(0))
        interleave(
            attn_units(3),
            outproj_units(1) + outproj_units(2),
            reserve=6,
        )
        flush_deferred()
        for u in outproj_units(3, act_copies=True):
            emit(u)
        flush_deferred()


def build_nc():
    if "nc" in _CACHE:
        return _CACHE["nc"]
    nc = bacc.Bacc("TRN2", target_bir_lowering=False, debug=False, num_devices=NCORES)
    xT = nc.dram_tensor("xT", (P, KSUB, S), BF, kind="ExternalInput").ap()
    wq = nc.dram_tensor("wq", (P, KSUB, DG), BF, kind="ExternalInput").ap()
    wk = nc.dram_tensor("wk", (P, KSUB, DG), BF, kind="ExternalInput").ap()
    wv = nc.dram_tensor("wv", (P, KSUB, DG), BF, kind="ExternalInput").ap()
    wo = nc.dram_tensor("wo", (P, DG // P, D), BF, kind="ExternalInput").ap()
    out = nc.dram_tensor("out", (S, D), BF, kind="ExternalOutput").ap()
    with tile.TileContext(nc) as tc:
        _mha_tile_kernel(tc, xT, wq, wk, wv, wo, out)
    nc.compile()
    _CACHE["nc"] = nc
    return nc


def make_in_maps(x, Wq, Wk, Wv, Wo):
    bf = ml_dtypes.bfloat16
    x = np.asarray(x, np.float32)
    in_maps = []
    for c in range(NCORES):
        b, g = c // GROUPS, c % GROUPS
        cols = slice(g * DG, (g + 1) * DG)

        def wslice(W):
            # [D, DG] -> [128, KSUB, DG] with [p, k, m] = W[k*128+p, m]
            return np.ascontiguousarray(
                np.asarray(W, np.float32)[:, cols]
                .reshape(KSUB, P, DG)
                .transpose(1, 0, 2)
                .astype(bf)
            )

        wo_c = np.ascontiguousarray(
            np.asarray(Wo, np.float32)[cols, :]
            .reshape(DG // P, P, D)
            .transpose(1, 0, 2)
            .astype(bf)
        )
        in_maps.append(
            {
                # x^T reshaped [128, KSUB, S] so one batched DMA per s-slice
                # lands contiguously per partition
                "xT": np.ascontiguousarray(
                    x[b].T.reshape(KSUB, P, S).transpose(1, 0, 2).astype(bf)
                ),
                "wq": wslice(Wq),
                "wk": wslice(Wk),
                "wv": wslice(Wv),
                "wo": wo_c,
            }
        )
    return in_maps


def kernel(x, Wq, Wk, Wv, Wo, bo):
    nc = build_nc()
    in_maps = make_in_maps(x, Wq, Wk, Wv, Wo)
    trace = bool(int(os.environ.get("MHA_TRACE", "0")))
    if trace:
        _install_ntff_hook()
    res = run_bass_kernel_spmd(
        nc, in_maps, core_ids=list(range(NCORES)), trace=trace,
        trace_cores=list(range(NCORES)) if trace else None,
    )
    _CACHE["last_results"] = res
    bo = np.asarray(bo, np.float32)
    out = np.zeros((B, S, D), np.float32)
    for c in range(NCORES):
        out[c // GROUPS] += np.asarray(res.results[c]["out"], np.float32)
    out += bo[None, None, :]
    return out

